# revision 13
# baseline (speedup 1.0000x reference)
"""Trainium2 Bass kernel for nn_ConcatSquashLinearSA.

Math (per sample b, S=1):
    gate = sigmoid(ctx @ Wg.T + bg)          [256]
    bias = ctx @ Wb.T                        [256]
    kv   = ctx @ Wkv.T                       [256]
    E    = outer(kv, kv)                     [256,256]
    A    = softmax_rows(E)
    att  = A / (1e-9 + colsum(A))
    out  = (x @ Wl.T + bl) @ (I + att) * gate + bias

which folds to a single big matmul per sample:
    P      = A * v,  v[e] = gate[e] / (1e-9 + colsum[e])
    W_eff2 = Wl.T @ P + Wl.T * gate          [256,256]   (tiny, on-device)
    b_fin  = bl*gate + bl @ P + bias         [256]
    out    = x @ W_eff2 + b_fin              [16384,256] (the only big op)

Sharding: data-parallel over batch, 2 samples per core across 8 cores.

Dataflow per core (memory-roofline play): the 2e-2 harness tolerance
allows bf16 streaming for both directions, halving HBM traffic vs fp32.
The host pre-transposes x to xT [256, 32768] bf16 so the PE does no
transposes: out is computed feature-major, outT[e, n] = sum_k
W_eff2[k, e] xT[k, n] + b_fin[e], with W_eff2 chunks as the stationary
operand and xT streamed.  b_fin lands on partitions (e-major), so the
bias-add + fp32->bf16 downcast is a single per-partition op, alternated
between the vector and scalar engines.  The host transposes outT back.

Per-core traffic: 16 MiB in + 16 MiB out (vs 32+32 fp32 baseline).
"""

import numpy as np

B, N, DIN, DOUT, DCTX = 16, 16384, 256, 256, 131
NCORES = 8
SPC = B // NCORES           # samples per core
ROWS = SPC * N              # x rows (= outT cols) per core
USE_FP8_X = True            # stream x as fp8e4m3 (rhs); W stays bf16
MACRO = 4096                # xT/outT columns per macro-tile
CHUNK = 512                 # PSUM bank width (fp32) per matmul group


def build_nc(rows=ROWS):
    import concourse.bass as bass
    import concourse.tile as tile
    from concourse import bacc, mybir
    from contextlib import ExitStack

    f32 = mybir.dt.float32
    bf16 = mybir.dt.bfloat16
    xdt = mybir.dt.float8e4 if USE_FP8_X else bf16
    AF = mybir.ActivationFunctionType
    AX = mybir.AxisListType
    OP = mybir.AluOpType

    n_macro_per_s = N // MACRO       # macro-tiles per sample
    n_chunks = MACRO // CHUNK

    nc = bacc.Bacc()
    xT_d = nc.declare_dram_parameter("xT", [256, rows], xdt, isOutput=False)
    ctxT_d = nc.declare_dram_parameter("ctxT", [256, SPC], f32, isOutput=False)
    wcatT_d = nc.declare_dram_parameter("wcatT", [256, 768], f32, isOutput=False)
    wlayer_d = nc.declare_dram_parameter("wlayer", [256, 256], f32, isOutput=False)
    wlayerT_d = nc.declare_dram_parameter("wlayerT", [256, 256], f32, isOutput=False)
    blr_d = nc.declare_dram_parameter("blayer_row", [1, 256], f32, isOutput=False)
    bgr_d = nc.declare_dram_parameter("bgate_row", [1, 256], f32, isOutput=False)
    blc_d = nc.declare_dram_parameter("blayer_col", [256, 1], f32, isOutput=False)
    onesr_d = nc.declare_dram_parameter("ones_row", [1, 128], f32, isOutput=False)
    onesc_d = nc.declare_dram_parameter("ones_col", [128, 1], f32, isOutput=False)
    outT_d = nc.declare_dram_parameter("outT", [256, rows], bf16, isOutput=True)

    with tile.TileContext(nc) as tc, ExitStack() as ctx:
        consts = ctx.enter_context(tc.tile_pool(name="consts", bufs=1))
        spool = ctx.enter_context(tc.tile_pool(name="scratch", bufs=2))
        perm = ctx.enter_context(tc.tile_pool(name="persample", bufs=1))
        pps = ctx.enter_context(tc.tile_pool(name="pps", bufs=3, space="PSUM"))
        pout = ctx.enter_context(tc.tile_pool(name="pout", bufs=4, space="PSUM"))
        xin = ctx.enter_context(tc.tile_pool(name="xin", bufs=4))
        osb = ctx.enter_context(tc.tile_pool(name="osb", bufs=3))

        def cload(name, dram_ap, shape, dt=f32):
            t = consts.tile(shape, dt, name=name, tag=name)
            nc.sync.dma_start(t, dram_ap)
            return t

        ctxT0 = cload("ctxT0", ctxT_d[0:128, :], [128, SPC])
        ctxT1 = cload("ctxT1", ctxT_d[128:256, :], [128, SPC])
        wcat0 = cload("wcat0", wcatT_d[0:128, :], [128, 768])
        wcat1 = cload("wcat1", wcatT_d[128:256, :], [128, 768])
        wl0 = cload("wl0", wlayer_d[0:128, :], [128, 256])
        wl1 = cload("wl1", wlayer_d[128:256, :], [128, 256])
        wlT = [cload("wlT0", wlayerT_d[0:128, :], [128, 256]),
               cload("wlT1", wlayerT_d[128:256, :], [128, 256])]
        blr = cload("blr", blr_d[:, :], [1, 256])
        bgr = cload("bgr", bgr_d[:, :], [1, 256])
        blc0 = cload("blc0", blc_d[0:128, :], [128, 1])
        blc1 = cload("blc1", blc_d[128:256, :], [128, 1])
        onesr = cload("onesr", onesr_d[:, :], [1, 128])
        onesc = cload("onesc", onesc_d[:, :], [128, 1])

        # Prologue: per-sample chains are serial, so emit each step for BOTH
        # samples back-to-back — engines execute in program order, and the
        # interleave lets the two dependency chains advance concurrently.
        weff = {}
        bfcol = {}
        svec, gate, Ab, P = {}, {}, {}, {}
        csb, rcs, vvec, Vb, GateB, bfin = {}, {}, {}, {}, {}, {}

        for s in range(SPC):
            # ---- ctx projections: [gate_pre | bias | kv] = ctx @ WcatT ----
            cat1 = pps.tile([1, 512], f32, name=f"cat1_{s}", tag="ps")
            nc.tensor.matmul(cat1, lhsT=ctxT0[:, s:s + 1], rhs=wcat0[:, 0:512],
                             start=True, stop=False)
            nc.tensor.matmul(cat1, lhsT=ctxT1[:, s:s + 1], rhs=wcat1[:, 0:512],
                             start=False, stop=True)
            cat2 = pps.tile([1, 256], f32, name=f"cat2_{s}", tag="ps")
            nc.tensor.matmul(cat2, lhsT=ctxT0[:, s:s + 1], rhs=wcat0[:, 512:768],
                             start=True, stop=False)
            nc.tensor.matmul(cat2, lhsT=ctxT1[:, s:s + 1], rhs=wcat1[:, 512:768],
                             start=False, stop=True)
            sv = spool.tile([1, 768], f32, name=f"svec{s}", tag=f"svec{s}")
            nc.vector.tensor_copy(sv[:, 0:512], cat1)
            nc.vector.tensor_copy(sv[:, 512:768], cat2)
            svec[s] = sv

        for s in range(SPC):
            gpre = spool.tile([1, 256], f32, name=f"gpre{s}", tag="gpre")
            nc.vector.tensor_add(gpre, svec[s][:, 0:256], bgr)
            g = spool.tile([1, 256], f32, name=f"gate{s}", tag=f"gate{s}")
            nc.scalar.activation(g, gpre, AF.Sigmoid)
            gate[s] = g

        # ---- E = outer(kv, kv); row softmax ----
        for s in range(SPC):
            for i in range(2):
                E = pps.tile([128, 256], f32, name=f"E{s}{i}", tag="ps")
                nc.tensor.matmul(E, lhsT=svec[s][0:1, 512 + 128 * i:640 + 128 * i],
                                 rhs=svec[s][0:1, 512:768], start=True, stop=True)
                negmx = spool.tile([128, 1], f32, name=f"negmx{s}{i}", tag="negmx")
                nc.vector.reduce_max(negmx, E, axis=AX.X, negate=True)
                expE = spool.tile([128, 256], f32, name=f"expE{s}{i}",
                                  tag=f"expE{s}{i}")
                nc.scalar.activation(expE, E, AF.Exp, bias=negmx)
                sm = spool.tile([128, 1], f32, name=f"sm{s}{i}", tag="sm")
                nc.vector.reduce_sum(sm, expE, axis=AX.X)
                rc = spool.tile([128, 1], f32, name=f"rc{s}{i}", tag="rc")
                nc.vector.reciprocal(rc, sm)
                A = spool.tile([128, 256], f32, name=f"A{s}{i}", tag=f"A{s}{i}")
                nc.vector.tensor_scalar_mul(A, expE, rc)
                Ab.setdefault(s, []).append(A)

        # ---- column sum; v = gate / (1e-9 + colsum) ----
        for s in range(SPC):
            cs = pps.tile([1, 256], f32, name=f"cs{s}", tag="ps")
            nc.tensor.matmul(cs, lhsT=onesc, rhs=Ab[s][0], start=True, stop=False)
            nc.tensor.matmul(cs, lhsT=onesc, rhs=Ab[s][1], start=False, stop=True)
            cb = spool.tile([1, 256], f32, name=f"csb{s}", tag=f"csb{s}")
            nc.vector.tensor_scalar_add(cb, cs, 1e-9)
            csb[s] = cb
        for s in range(SPC):
            rr = spool.tile([1, 256], f32, name=f"rcs{s}", tag=f"rcs{s}")
            nc.vector.reciprocal(rr, csb[s])
            rcs[s] = rr
        for s in range(SPC):
            vv = spool.tile([1, 256], f32, name=f"vvec{s}", tag=f"vvec{s}")
            nc.vector.tensor_mul(vv, rcs[s], gate[s])
            vvec[s] = vv

        # ---- broadcast v and gate to [128,256] via rank-1 matmul ----
        for s in range(SPC):
            vbp = pps.tile([128, 256], f32, name=f"vbp{s}", tag="ps")
            nc.tensor.matmul(vbp, lhsT=onesr, rhs=vvec[s], start=True, stop=True)
            vb = spool.tile([128, 256], f32, name=f"Vb{s}", tag=f"Vb{s}")
            nc.vector.tensor_copy(vb, vbp)
            Vb[s] = vb
            gbp = pps.tile([128, 256], f32, name=f"gbp{s}", tag="ps")
            nc.tensor.matmul(gbp, lhsT=onesr, rhs=gate[s], start=True, stop=True)
            gb = spool.tile([128, 256], f32, name=f"GateB{s}", tag=f"GateB{s}")
            nc.vector.tensor_copy(gb, gbp)
            GateB[s] = gb
        for s in range(SPC):
            Ps = []
            for i in range(2):
                Pi = spool.tile([128, 256], f32, name=f"P{s}{i}", tag=f"P{s}{i}")
                nc.vector.tensor_mul(Pi, Ab[s][i], Vb[s])
                Ps.append(Pi)
            P[s] = Ps

        # ---- W_eff2 = Wl.T @ P + Wl.T * gate  (bf16, [k, e] layout) ----
        for s in range(SPC):
            for j in range(2):
                wp = pps.tile([128, 256], f32, name=f"wp{s}{j}", tag="ps")
                nc.tensor.matmul(wp, lhsT=wl0[:, 128 * j:128 * (j + 1)],
                                 rhs=P[s][0], start=True, stop=False)
                nc.tensor.matmul(wp, lhsT=wl1[:, 128 * j:128 * (j + 1)],
                                 rhs=P[s][1], start=False, stop=True)
                tmpW = spool.tile([128, 256], f32, name=f"tmpW{s}{j}", tag="tmpW")
                nc.vector.tensor_mul(tmpW, wlT[j], GateB[s])
                wsb = perm.tile([128, 256], bf16, name=f"weff{s}{j}",
                                tag=f"weff{s}{j}")
                nc.vector.tensor_add(wsb, wp, tmpW)
                weff[(s, j)] = wsb

        # ---- b_fin = bl*gate + bl @ P + bias, as [128,1] columns ----
        for s in range(SPC):
            qp = pps.tile([1, 256], f32, name=f"qp{s}", tag="ps")
            nc.tensor.matmul(qp, lhsT=blc0, rhs=P[s][0], start=True, stop=False)
            nc.tensor.matmul(qp, lhsT=blc1, rhs=P[s][1], start=False, stop=True)
            tb = spool.tile([1, 256], f32, name=f"tb{s}", tag="tb")
            nc.vector.tensor_mul(tb, blr, gate[s])
            tb2 = spool.tile([1, 256], f32, name=f"tb2{s}", tag="tb2")
            nc.vector.tensor_add(tb2, tb, qp)
            bf = spool.tile([1, 256], f32, name=f"bfin{s}", tag=f"bfin{s}")
            nc.vector.tensor_add(bf, tb2, svec[s][:, 256:512])
            bfin[s] = bf
        for s in range(SPC):
            for h in range(2):
                bfp = pps.tile([128, 1], f32, name=f"bfp{s}{h}", tag="ps")
                nc.tensor.matmul(bfp, lhsT=bfin[s][0:1, 128 * h:128 * (h + 1)],
                                 rhs=onesr[0:1, 0:1], start=True, stop=True)
                bcol = perm.tile([128, 1], f32, name=f"bfcol{s}{h}",
                                 tag=f"bfcol{s}{h}")
                nc.vector.tensor_copy(bcol, bfp)
                bfcol[(s, h)] = bcol

        # ---- main loop: outT[e, n] = sum_k W_eff2[k, e] xT[k, n] + b_fin[e]
        HALF = MACRO // 2
        for s in range(SPC):
            for m in range(n_macro_per_s):
                off = N * s + MACRO * m
                xk = xin.tile([128, 2 * MACRO], xdt, name="xk", tag="xk")
                nc.sync.dma_start(xk[:, 0:MACRO], xT_d[0:128, off:off + MACRO])
                nc.sync.dma_start(xk[:, MACRO:2 * MACRO],
                                  xT_d[128:256, off:off + MACRO])
                o0 = osb.tile([128, MACRO], bf16, name="o0", tag="o0")
                o1 = osb.tile([128, MACRO], bf16, name="o1", tag="o1")
                for c in range(n_chunks):
                    lo = CHUNK * c
                    for ec in range(2):
                        ps = pout.tile([128, CHUNK], f32, name="ps", tag="ps")
                        nc.tensor.matmul(ps,
                                         lhsT=weff[(s, 0)][:, 128 * ec:128 * ec + 128],
                                         rhs=xk[:, lo:lo + CHUNK],
                                         start=True, stop=False)
                        nc.tensor.matmul(ps,
                                         lhsT=weff[(s, 1)][:, 128 * ec:128 * ec + 128],
                                         rhs=xk[:, MACRO + lo:MACRO + lo + CHUNK],
                                         start=False, stop=True)
                        if ec == 0:
                            nc.vector.tensor_scalar_add(o0[:, lo:lo + CHUNK],
                                                        ps, bfcol[(s, 0)])
                        else:
                            nc.scalar.activation(o1[:, lo:lo + CHUNK], ps,
                                                 AF.Identity, bias=bfcol[(s, 1)])
                    # stream out each quarter-macro as soon as its chunks drain
                    QUAR = MACRO // 4
                    if CHUNK * (c + 1) % QUAR == 0:
                        lo2 = CHUNK * (c + 1) - QUAR
                        nc.gpsimd.dma_start(
                            outT_d[0:128, off + lo2:off + lo2 + QUAR],
                            o0[:, lo2:lo2 + QUAR])
                        nc.gpsimd.dma_start(
                            outT_d[128:256, off + lo2:off + lo2 + QUAR],
                            o1[:, lo2:lo2 + QUAR])

    nc.finalize()
    return nc


def prep_host_inputs(ctx, x, W_layer, b_layer, W_bias, W_gate, b_gate, W_kv,
                     rows=ROWS):
    """Build the per-core in_maps (host-side sharding + constant re-layout)."""
    import ml_dtypes
    bf16 = ml_dtypes.bfloat16
    xdt = ml_dtypes.float8_e4m3 if USE_FP8_X else bf16

    ctx = np.asarray(ctx, np.float32)
    x = np.asarray(x, np.float32)
    W_layer = np.asarray(W_layer, np.float32)
    b_layer = np.asarray(b_layer, np.float32)
    W_bias = np.asarray(W_bias, np.float32)
    W_gate = np.asarray(W_gate, np.float32)
    b_gate = np.asarray(b_gate, np.float32)
    W_kv = np.asarray(W_kv, np.float32)

    wcatT = np.zeros((256, 768), np.float32)
    wcatT[:DCTX, 0:256] = W_gate.T
    wcatT[:DCTX, 256:512] = W_bias.T
    wcatT[:DCTX, 512:768] = W_kv.T
    shared = {
        "wcatT": wcatT,
        "wlayer": np.ascontiguousarray(W_layer),
        "wlayerT": np.ascontiguousarray(W_layer.T),
        "blayer_row": b_layer.reshape(1, 256).copy(),
        "bgate_row": b_gate.reshape(1, 256).copy(),
        "blayer_col": b_layer.reshape(256, 1).copy(),
        "ones_row": np.ones((1, 128), np.float32),
        "ones_col": np.ones((128, 1), np.float32),
    }
    in_maps = []
    for c in range(NCORES):
        ctxT = np.zeros((256, SPC), np.float32)
        for k in range(SPC):
            ctxT[:DCTX, k] = ctx[SPC * c + k, 0]
        xT = np.empty((256, rows), xdt)
        for k in range(SPC):
            xT[:, N * k:N * (k + 1)] = x[SPC * c + k].T
        in_maps.append({"xT": xT, "ctxT": ctxT, **shared})
    return in_maps


def kernel(ctx, x, W_layer, b_layer, W_bias, W_gate, b_gate, W_kv):
    from concourse.bass_utils import run_bass_kernel_spmd

    nc = build_nc(ROWS)
    in_maps = prep_host_inputs(ctx, x, W_layer, b_layer, W_bias, W_gate,
                               b_gate, W_kv)
    res = run_bass_kernel_spmd(nc, in_maps, core_ids=list(range(NCORES)))
    out = np.empty((B, N, DOUT), np.float32)
    for c in range(NCORES):
        oT = np.asarray(res.results[c]["outT"], dtype=np.float32)
        for k in range(SPC):
            out[SPC * c + k] = oT[:, N * k:N * (k + 1)].T
    return out


# revision 15
# speedup vs baseline: 1.0759x; 1.0759x over previous
"""Trainium2 Bass kernel for nn_ConcatSquashLinearSA.

Math (per sample b, S=1):
    gate = sigmoid(ctx @ Wg.T + bg)          [256]
    bias = ctx @ Wb.T                        [256]
    kv   = ctx @ Wkv.T                       [256]
    E    = outer(kv, kv)                     [256,256]
    A    = softmax_rows(E)
    att  = A / (1e-9 + colsum(A))
    out  = (x @ Wl.T + bl) @ (I + att) * gate + bias

which folds to a single big matmul per sample:
    P      = A * v,  v[e] = gate[e] / (1e-9 + colsum[e])
    W_eff2 = Wl.T @ P + Wl.T * gate          [256,256]   (tiny, on-device)
    b_fin  = bl*gate + bl @ P + bias         [256]
    out    = x @ W_eff2 + b_fin              [16384,256] (the only big op)

Sharding: data-parallel over batch, 2 samples per core across 8 cores.

Dataflow per core (memory-roofline play): the 2e-2 harness tolerance
allows low-precision streaming — x goes in as fp8e4m3 (PE allows mixed
fp8 rhs x bf16 lhsT), the output leaves as bf16, W_eff2 is bf16
(measured rel err 1.58e-2 vs the 2e-2 gate).  The host pre-transposes
x to xT [256, 32768] so the PE does no transposes: out is computed
feature-major, outT[e, n] = sum_k W_eff2[k, e] xT[k, n] + b_fin[e],
with W_eff2 chunks as the stationary operand and xT streamed.  b_fin
lands on partitions (e-major), so the bias-add + fp32->bf16 downcast
is a single per-partition op, alternated between the vector and scalar
engines.  The host transposes outT back.  In-DMAs issue on the sync
HWDGE ring, out-DMAs on the gpsimd SWDGE ring (two independent issue
streams); outs stream at half-macro granularity.

Per-core traffic: 8 MiB in + 16 MiB out (vs 32+32 fp32 baseline).
"""

import numpy as np

B, N, DIN, DOUT, DCTX = 16, 16384, 256, 256, 131
NCORES = 8
SPC = B // NCORES           # samples per core
ROWS = SPC * N              # x rows (= outT cols) per core
USE_FP8_X = True            # stream x as fp8e4m3 (rhs); W stays bf16
MACRO = 4096                # xT/outT columns per macro-tile
CHUNK = 512                 # PSUM bank width (fp32) per matmul group


def build_nc(rows=ROWS):
    import concourse.bass as bass
    import concourse.tile as tile
    from concourse import bacc, mybir
    from contextlib import ExitStack

    f32 = mybir.dt.float32
    bf16 = mybir.dt.bfloat16
    xdt = mybir.dt.float8e4 if USE_FP8_X else bf16
    AF = mybir.ActivationFunctionType
    AX = mybir.AxisListType
    OP = mybir.AluOpType

    n_macro_per_s = N // MACRO       # macro-tiles per sample
    n_chunks = MACRO // CHUNK

    nc = bacc.Bacc()
    xT_d = nc.declare_dram_parameter("xT", [256, rows], xdt, isOutput=False)
    ctxT_d = nc.declare_dram_parameter("ctxT", [256, SPC], f32, isOutput=False)
    wcatT_d = nc.declare_dram_parameter("wcatT", [256, 768], f32, isOutput=False)
    wlayer_d = nc.declare_dram_parameter("wlayer", [256, 256], f32, isOutput=False)
    wlayerT_d = nc.declare_dram_parameter("wlayerT", [256, 256], f32, isOutput=False)
    blr_d = nc.declare_dram_parameter("blayer_row", [1, 256], f32, isOutput=False)
    bgr_d = nc.declare_dram_parameter("bgate_row", [1, 256], f32, isOutput=False)
    blc_d = nc.declare_dram_parameter("blayer_col", [256, 1], f32, isOutput=False)
    onesr_d = nc.declare_dram_parameter("ones_row", [1, 128], f32, isOutput=False)
    onesc_d = nc.declare_dram_parameter("ones_col", [128, 1], f32, isOutput=False)
    outT_d = nc.declare_dram_parameter("outT", [256, rows], bf16, isOutput=True)

    with tile.TileContext(nc) as tc, ExitStack() as ctx:
        consts = ctx.enter_context(tc.tile_pool(name="consts", bufs=1))
        spool = ctx.enter_context(tc.tile_pool(name="scratch", bufs=2))
        perm = ctx.enter_context(tc.tile_pool(name="persample", bufs=1))
        pps = ctx.enter_context(tc.tile_pool(name="pps", bufs=3, space="PSUM"))
        pout = ctx.enter_context(tc.tile_pool(name="pout", bufs=4, space="PSUM"))
        xin = ctx.enter_context(tc.tile_pool(name="xin", bufs=4))
        osb = ctx.enter_context(tc.tile_pool(name="osb", bufs=3))

        def cload(name, dram_ap, shape, dt=f32):
            t = consts.tile(shape, dt, name=name, tag=name)
            nc.sync.dma_start(t, dram_ap)
            return t

        ctxT0 = cload("ctxT0", ctxT_d[0:128, :], [128, SPC])
        ctxT1 = cload("ctxT1", ctxT_d[128:256, :], [128, SPC])
        wcat0 = cload("wcat0", wcatT_d[0:128, :], [128, 768])
        wcat1 = cload("wcat1", wcatT_d[128:256, :], [128, 768])
        wl0 = cload("wl0", wlayer_d[0:128, :], [128, 256])
        wl1 = cload("wl1", wlayer_d[128:256, :], [128, 256])
        wlT = [cload("wlT0", wlayerT_d[0:128, :], [128, 256]),
               cload("wlT1", wlayerT_d[128:256, :], [128, 256])]
        blr = cload("blr", blr_d[:, :], [1, 256])
        bgr = cload("bgr", bgr_d[:, :], [1, 256])
        blc0 = cload("blc0", blc_d[0:128, :], [128, 1])
        blc1 = cload("blc1", blc_d[128:256, :], [128, 1])
        onesr = cload("onesr", onesr_d[:, :], [1, 128])
        onesc = cload("onesc", onesc_d[:, :], [128, 1])

        # Prologue: per-sample chains are serial, so emit each step for BOTH
        # samples back-to-back — engines execute in program order, and the
        # interleave lets the two dependency chains advance concurrently.
        weff = {}
        bfcol = {}
        svec, gate, Ab, P = {}, {}, {}, {}
        csb, rcs, vvec, Vb, GateB, bfin = {}, {}, {}, {}, {}, {}

        for s in range(SPC):
            # ---- ctx projections: [gate_pre | bias | kv] = ctx @ WcatT ----
            cat1 = pps.tile([1, 512], f32, name=f"cat1_{s}", tag="ps")
            nc.tensor.matmul(cat1, lhsT=ctxT0[:, s:s + 1], rhs=wcat0[:, 0:512],
                             start=True, stop=False)
            nc.tensor.matmul(cat1, lhsT=ctxT1[:, s:s + 1], rhs=wcat1[:, 0:512],
                             start=False, stop=True)
            cat2 = pps.tile([1, 256], f32, name=f"cat2_{s}", tag="ps")
            nc.tensor.matmul(cat2, lhsT=ctxT0[:, s:s + 1], rhs=wcat0[:, 512:768],
                             start=True, stop=False)
            nc.tensor.matmul(cat2, lhsT=ctxT1[:, s:s + 1], rhs=wcat1[:, 512:768],
                             start=False, stop=True)
            sv = spool.tile([1, 768], f32, name=f"svec{s}", tag=f"svec{s}")
            nc.vector.tensor_copy(sv[:, 0:512], cat1)
            nc.vector.tensor_copy(sv[:, 512:768], cat2)
            svec[s] = sv

        for s in range(SPC):
            gpre = spool.tile([1, 256], f32, name=f"gpre{s}", tag="gpre")
            nc.vector.tensor_add(gpre, svec[s][:, 0:256], bgr)
            g = spool.tile([1, 256], f32, name=f"gate{s}", tag=f"gate{s}")
            nc.scalar.activation(g, gpre, AF.Sigmoid)
            gate[s] = g

        # ---- E = outer(kv, kv); row softmax ----
        for s in range(SPC):
            for i in range(2):
                E = pps.tile([128, 256], f32, name=f"E{s}{i}", tag="ps")
                nc.tensor.matmul(E, lhsT=svec[s][0:1, 512 + 128 * i:640 + 128 * i],
                                 rhs=svec[s][0:1, 512:768], start=True, stop=True)
                negmx = spool.tile([128, 1], f32, name=f"negmx{s}{i}", tag="negmx")
                nc.vector.reduce_max(negmx, E, axis=AX.X, negate=True)
                expE = spool.tile([128, 256], f32, name=f"expE{s}{i}",
                                  tag=f"expE{s}{i}")
                nc.scalar.activation(expE, E, AF.Exp, bias=negmx)
                sm = spool.tile([128, 1], f32, name=f"sm{s}{i}", tag="sm")
                nc.vector.reduce_sum(sm, expE, axis=AX.X)
                rc = spool.tile([128, 1], f32, name=f"rc{s}{i}", tag="rc")
                nc.vector.reciprocal(rc, sm)
                A = spool.tile([128, 256], f32, name=f"A{s}{i}", tag=f"A{s}{i}")
                nc.vector.tensor_scalar_mul(A, expE, rc)
                Ab.setdefault(s, []).append(A)

        # ---- column sum; v = gate / (1e-9 + colsum) ----
        for s in range(SPC):
            cs = pps.tile([1, 256], f32, name=f"cs{s}", tag="ps")
            nc.tensor.matmul(cs, lhsT=onesc, rhs=Ab[s][0], start=True, stop=False)
            nc.tensor.matmul(cs, lhsT=onesc, rhs=Ab[s][1], start=False, stop=True)
            cb = spool.tile([1, 256], f32, name=f"csb{s}", tag=f"csb{s}")
            nc.vector.tensor_scalar_add(cb, cs, 1e-9)
            csb[s] = cb
        for s in range(SPC):
            rr = spool.tile([1, 256], f32, name=f"rcs{s}", tag=f"rcs{s}")
            nc.vector.reciprocal(rr, csb[s])
            rcs[s] = rr
        for s in range(SPC):
            vv = spool.tile([1, 256], f32, name=f"vvec{s}", tag=f"vvec{s}")
            nc.vector.tensor_mul(vv, rcs[s], gate[s])
            vvec[s] = vv

        # ---- broadcast v and gate to [128,256] via rank-1 matmul ----
        for s in range(SPC):
            vbp = pps.tile([128, 256], f32, name=f"vbp{s}", tag="ps")
            nc.tensor.matmul(vbp, lhsT=onesr, rhs=vvec[s], start=True, stop=True)
            vb = spool.tile([128, 256], f32, name=f"Vb{s}", tag=f"Vb{s}")
            nc.vector.tensor_copy(vb, vbp)
            Vb[s] = vb
            gbp = pps.tile([128, 256], f32, name=f"gbp{s}", tag="ps")
            nc.tensor.matmul(gbp, lhsT=onesr, rhs=gate[s], start=True, stop=True)
            gb = spool.tile([128, 256], f32, name=f"GateB{s}", tag=f"GateB{s}")
            nc.vector.tensor_copy(gb, gbp)
            GateB[s] = gb
        for s in range(SPC):
            Ps = []
            for i in range(2):
                Pi = spool.tile([128, 256], f32, name=f"P{s}{i}", tag=f"P{s}{i}")
                nc.vector.tensor_mul(Pi, Ab[s][i], Vb[s])
                Ps.append(Pi)
            P[s] = Ps

        # ---- W_eff2 = Wl.T @ P + Wl.T * gate  (bf16, [k, e] layout) ----
        for s in range(SPC):
            for j in range(2):
                wp = pps.tile([128, 256], f32, name=f"wp{s}{j}", tag="ps")
                nc.tensor.matmul(wp, lhsT=wl0[:, 128 * j:128 * (j + 1)],
                                 rhs=P[s][0], start=True, stop=False)
                nc.tensor.matmul(wp, lhsT=wl1[:, 128 * j:128 * (j + 1)],
                                 rhs=P[s][1], start=False, stop=True)
                tmpW = spool.tile([128, 256], f32, name=f"tmpW{s}{j}", tag="tmpW")
                nc.vector.tensor_mul(tmpW, wlT[j], GateB[s])
                wsb = perm.tile([128, 256], bf16, name=f"weff{s}{j}",
                                tag=f"weff{s}{j}")
                nc.vector.tensor_add(wsb, wp, tmpW)
                weff[(s, j)] = wsb

        # ---- b_fin = bl*gate + bl @ P + bias, as [128,1] columns ----
        for s in range(SPC):
            qp = pps.tile([1, 256], f32, name=f"qp{s}", tag="ps")
            nc.tensor.matmul(qp, lhsT=blc0, rhs=P[s][0], start=True, stop=False)
            nc.tensor.matmul(qp, lhsT=blc1, rhs=P[s][1], start=False, stop=True)
            tb = spool.tile([1, 256], f32, name=f"tb{s}", tag="tb")
            nc.vector.tensor_mul(tb, blr, gate[s])
            tb2 = spool.tile([1, 256], f32, name=f"tb2{s}", tag="tb2")
            nc.vector.tensor_add(tb2, tb, qp)
            bf = spool.tile([1, 256], f32, name=f"bfin{s}", tag=f"bfin{s}")
            nc.vector.tensor_add(bf, tb2, svec[s][:, 256:512])
            bfin[s] = bf
        for s in range(SPC):
            for h in range(2):
                bfp = pps.tile([128, 1], f32, name=f"bfp{s}{h}", tag="ps")
                nc.tensor.matmul(bfp, lhsT=bfin[s][0:1, 128 * h:128 * (h + 1)],
                                 rhs=onesr[0:1, 0:1], start=True, stop=True)
                bcol = perm.tile([128, 1], f32, name=f"bfcol{s}{h}",
                                 tag=f"bfcol{s}{h}")
                nc.vector.tensor_copy(bcol, bfp)
                bfcol[(s, h)] = bcol

        # ---- main loop: outT[e, n] = sum_k W_eff2[k, e] xT[k, n] + b_fin[e]
        HALF = MACRO // 2
        for s in range(SPC):
            for m in range(n_macro_per_s):
                off = N * s + MACRO * m
                xk = xin.tile([128, 2 * MACRO], xdt, name="xk", tag="xk")
                nc.sync.dma_start(xk[:, 0:MACRO], xT_d[0:128, off:off + MACRO])
                nc.sync.dma_start(xk[:, MACRO:2 * MACRO],
                                  xT_d[128:256, off:off + MACRO])
                o0 = osb.tile([128, MACRO], bf16, name="o0", tag="o0")
                o1 = osb.tile([128, MACRO], bf16, name="o1", tag="o1")
                for c in range(n_chunks):
                    lo = CHUNK * c
                    for ec in range(2):
                        ps = pout.tile([128, CHUNK], f32, name="ps", tag="ps")
                        nc.tensor.matmul(ps,
                                         lhsT=weff[(s, 0)][:, 128 * ec:128 * ec + 128],
                                         rhs=xk[:, lo:lo + CHUNK],
                                         start=True, stop=False)
                        nc.tensor.matmul(ps,
                                         lhsT=weff[(s, 1)][:, 128 * ec:128 * ec + 128],
                                         rhs=xk[:, MACRO + lo:MACRO + lo + CHUNK],
                                         start=False, stop=True)
                        if ec == 0:
                            nc.vector.tensor_scalar_add(o0[:, lo:lo + CHUNK],
                                                        ps, bfcol[(s, 0)])
                        else:
                            nc.scalar.activation(o1[:, lo:lo + CHUNK], ps,
                                                 AF.Identity, bias=bfcol[(s, 1)])
                    # stream out each half-macro as soon as its chunks drain
                    if CHUNK * (c + 1) == HALF:
                        nc.gpsimd.dma_start(outT_d[0:128, off:off + HALF],
                                            o0[:, 0:HALF])
                        nc.gpsimd.dma_start(outT_d[128:256, off:off + HALF],
                                            o1[:, 0:HALF])
                nc.gpsimd.dma_start(outT_d[0:128, off + HALF:off + MACRO],
                                    o0[:, HALF:MACRO])
                nc.gpsimd.dma_start(outT_d[128:256, off + HALF:off + MACRO],
                                    o1[:, HALF:MACRO])

    nc.finalize()
    return nc


def prep_host_inputs(ctx, x, W_layer, b_layer, W_bias, W_gate, b_gate, W_kv,
                     rows=ROWS):
    """Build the per-core in_maps (host-side sharding + constant re-layout)."""
    import ml_dtypes
    bf16 = ml_dtypes.bfloat16
    xdt = ml_dtypes.float8_e4m3 if USE_FP8_X else bf16

    ctx = np.asarray(ctx, np.float32)
    x = np.asarray(x, np.float32)
    W_layer = np.asarray(W_layer, np.float32)
    b_layer = np.asarray(b_layer, np.float32)
    W_bias = np.asarray(W_bias, np.float32)
    W_gate = np.asarray(W_gate, np.float32)
    b_gate = np.asarray(b_gate, np.float32)
    W_kv = np.asarray(W_kv, np.float32)

    wcatT = np.zeros((256, 768), np.float32)
    wcatT[:DCTX, 0:256] = W_gate.T
    wcatT[:DCTX, 256:512] = W_bias.T
    wcatT[:DCTX, 512:768] = W_kv.T
    shared = {
        "wcatT": wcatT,
        "wlayer": np.ascontiguousarray(W_layer),
        "wlayerT": np.ascontiguousarray(W_layer.T),
        "blayer_row": b_layer.reshape(1, 256).copy(),
        "bgate_row": b_gate.reshape(1, 256).copy(),
        "blayer_col": b_layer.reshape(256, 1).copy(),
        "ones_row": np.ones((1, 128), np.float32),
        "ones_col": np.ones((128, 1), np.float32),
    }
    in_maps = []
    for c in range(NCORES):
        ctxT = np.zeros((256, SPC), np.float32)
        for k in range(SPC):
            ctxT[:DCTX, k] = ctx[SPC * c + k, 0]
        xT = np.empty((256, rows), xdt)
        for k in range(SPC):
            xT[:, N * k:N * (k + 1)] = x[SPC * c + k].T
        in_maps.append({"xT": xT, "ctxT": ctxT, **shared})
    return in_maps


def kernel(ctx, x, W_layer, b_layer, W_bias, W_gate, b_gate, W_kv):
    from concourse.bass_utils import run_bass_kernel_spmd

    nc = build_nc(ROWS)
    in_maps = prep_host_inputs(ctx, x, W_layer, b_layer, W_bias, W_gate,
                               b_gate, W_kv)
    res = run_bass_kernel_spmd(nc, in_maps, core_ids=list(range(NCORES)))
    out = np.empty((B, N, DOUT), np.float32)
    for c in range(NCORES):
        oT = np.asarray(res.results[c]["outT"], dtype=np.float32)
        for k in range(SPC):
            out[SPC * c + k] = oT[:, N * k:N * (k + 1)].T
    return out
